# revision 22
# baseline (speedup 1.0000x reference)
"""Trainium2 Bass kernel for BiochemicalDynamics.

Reference computation (f32):
    Ax    = A @ x                                   # [N, DIM]
    s     = R * rowsum(x * Ax)                      # [N, 1]
    out   = F - B*x - s                             # [N, DIM]

Design (v6): compute Y = (A_c @ x)^T on the TensorEngine directly.
Each core holds A_c = A[rows_c, :] shipped as fp8(e4m3) A_c^T tiles
("bt").  For each 128-row j-block a matmul with stationary xs[jblock]
(fp8 x) and moving bt[jblock] accumulates Y[d, i] += sum_j x[j,d]*A[i,j]
in PSUM.  This keeps the per-element A work on the PE (fastest engine)
instead of the DVE (the old bottleneck) and halves HBM traffic vs fp16.

Column-tiling: even j-blocks run at tile_position (0,0) -> PSUM
partitions 0..63, odd j-blocks at (0,64) -> partitions 64..127; the two
streams execute concurrently in the PE array.  The partition split is
free: the final dot already sums over partitions (via a -1s-stationary
matmul).

Two pipelines over the output-column halves (i in [0,512), [512,1024)):
each half streams its ~4.2MB of bt, accumulates Y_h, then
D_h = (R x^T (.) Y_h) on the DVE, a reduction matmul into OutP (seeded
early with -B*x^T via a -B*I stationary matmul), ScalarE Copies (+F
bias) and output DMAs.  Half 0's epilogue overlaps half 1's stream;
half 0's reduction matmul is emitted a few pairs into half 1's stream
so the PE never idles waiting on the DVE.  Half 1's trailing DMA
chunks ramp down to 256KB and its epilogue is split in two 256-column
pieces so the ScalarE copy and the out DMA pipeline.

A burst of warm-up matmuls on a memset scratch tile runs during the
otherwise-dead framework preamble (~5us) so the PE's HAM clock gate is
already at 8/8 (2.4 GHz) when the real matmul stream begins.

Sharding: row-shard A across the 8 cores; every core gets the full x
(host-side replication).  No cross-core communication.
"""

import sys

import numpy as np

for _p in ("/opt/trn_rl_repo", "/root/.axon_site/_ro/trn_rl_repo"):
    if _p not in sys.path:
        sys.path.append(_p)

N = 8192
DIM = 64
NCORES = 8
ROWS = N // NCORES       # 1024 rows of A (and output) per core

F_CONST = 1.0
B_CONST = 0.1
R_CONST = 0.01

P = 128                  # SBUF partitions
NBLK = N // P            # 64 j-blocks
HALF = 512               # output-column half width
NH = ROWS // HALF        # 2 halves
HBYTES = NBLK * HALF     # fp8 bytes per half per partition

# bt DMA chunks per half, in j-blocks (block-tile = 64KB fp8).
BT_CHUNKS = [
    [4, 4, 8, 8, 8, 8, 8, 8, 8],           # half 0
    [4, 4, 8, 8, 8, 8, 8, 8, 4, 2, 1, 1],  # half 1: ramp down at the end
]
assert all(sum(c) == NBLK for c in BT_CHUNKS)

N_WARM = 16              # warm-up matmuls (~3.4us cold) to trip HAM

_CACHE = {}


def _build_nc():
    import concourse.mybir as mybir
    import concourse.tile as tile
    from concourse import bacc

    f32 = mybir.dt.float32
    bf16 = mybir.dt.bfloat16
    f8 = mybir.dt.float8e4

    nc = bacc.Bacc(
        trn_type="TRN2", target_bir_lowering=False, debug=False, num_devices=NCORES
    )

    # A^T blocks, fp8: bt[p, h*HBYTES + b*HALF + i'] = A[rows_c[HALF*h+i'], 128b+p]
    bt = nc.dram_tensor("bt", [P, NH * HBYTES], f8, kind="ExternalInput")
    # x stationaries, fp8: xs[p, 64*b + d] = x[128*b + p, d]
    xs = nc.dram_tensor("xs", [P, NBLK * DIM], f8, kind="ExternalInput")
    # R*xloc^T bf16 (duplicated into both partition halves on-device)
    xtr = nc.dram_tensor("xtr", [DIM, ROWS], bf16, kind="ExternalInput")
    # xloc^T bf16 (moving operand of the -B*x seed matmul)
    xtb = nc.dram_tensor("xtb", [DIM, ROWS], bf16, kind="ExternalInput")
    # -1s [128, 64] and -B*I64, bf16 stationaries
    wneg = nc.dram_tensor("wneg", [P, DIM], bf16, kind="ExternalInput")
    wbi = nc.dram_tensor("wbi", [DIM, DIM], bf16, kind="ExternalInput")
    # out^T bf16: out[d, i] = F - B*xloc[i, d] - s_i
    out = nc.dram_tensor("out", [DIM, ROWS], bf16, kind="ExternalOutput")

    mult = mybir.AluOpType.mult

    with tile.TileContext(nc) as tc:
        with (
            tc.tile_pool(name="big", bufs=1) as big,
            tc.tile_pool(name="small", bufs=1) as small,
            tc.tile_pool(name="psum", bufs=1, space="PSUM") as psum_pool,
        ):
            # --- PE warm-up on a memset scratch tile (no input deps) ---
            scr = small.tile([P, 256], f32)
            nc.vector.memset(scr[:], 1.0)
            warm_ps = psum_pool.tile([DIM, 256], f32, tag="warm")
            for _ in range(N_WARM):
                nc.tensor.matmul(
                    warm_ps[:], scr[:, :DIM], scr[:], start=True, stop=True
                )

            # --- x-side loads on the Scalar (ACT) HWDGE ring ---
            wbi_sb = small.tile([DIM, DIM], bf16)
            nc.scalar.dma_start(out=wbi_sb[:], in_=wbi[:])
            xtb_sb = small.tile([DIM, ROWS], bf16)
            nc.scalar.dma_start(out=xtb_sb[:], in_=xtb[:])
            xs_sb = small.tile([P, NBLK * DIM], f8)
            for o, w in ((0, 32 * DIM), (32 * DIM, 32 * DIM)):
                nc.scalar.dma_start(out=xs_sb[:, o : o + w], in_=xs[:, o : o + w])
            # xtr2 = [R*x^T; R*x^T]: one HBM load + an SBUF->SBUF
            # duplicate (saves 128KB of HBM on the critical stream).
            xtr2_sb = small.tile([P, ROWS], bf16)
            nc.scalar.dma_start(out=xtr2_sb[:DIM, :], in_=xtr[:])
            nc.scalar.dma_start(out=xtr2_sb[DIM:, :], in_=xtr2_sb[:DIM, :])
            wneg_sb = small.tile([P, DIM], bf16)
            nc.scalar.dma_start(out=wneg_sb[:], in_=wneg[:])

            # --- A^T stream, all on the Sync HWDGE ring (a second busy
            # queue degrades both; measured). ---
            bt_sb = big.tile([P, NH * HBYTES], f8)
            for h in range(NH):
                boff = 0
                for nb in BT_CHUNKS[h]:
                    o = h * HBYTES + boff * HALF
                    w = nb * HALF
                    nc.sync.dma_start(out=bt_sb[:, o : o + w], in_=bt[:, o : o + w])
                    boff += nb

            # Output accumulator [64, 1024] f32 (2 PSUM banks).
            outp = psum_pool.tile([DIM, ROWS], f32, tag="outp")
            # Seeds: OutP = -B * xloc^T, one 256-col piece per matmul so
            # each piece forms its own accumulation group with its
            # reduction matmul (start=True clears).
            PIECE = HALF // 2
            for q in range(ROWS // PIECE):
                nc.tensor.matmul(
                    outp[:, q * PIECE : (q + 1) * PIECE],
                    wbi_sb[:],
                    xtb_sb[:, q * PIECE : (q + 1) * PIECE],
                    start=True, stop=False,
                )

            o_sb = small.tile([DIM, ROWS], bf16)
            ys = [
                psum_pool.tile([P, HALF], f32, tag=f"y{h}", name=f"y{h}")
                for h in range(NH)
            ]
            d_sbs = [
                small.tile([P, HALF], bf16, tag=f"d{h}", name=f"d{h}")
                for h in range(NH)
            ]

            def emit_y_mm(h, k):
                for half, b in ((0, 2 * k), (1, 2 * k + 1)):
                    nc.tensor.matmul(
                        ys[h][half * DIM : (half + 1) * DIM, :],
                        xs_sb[:, b * DIM : (b + 1) * DIM],
                        bt_sb[
                            :,
                            h * HBYTES + b * HALF : h * HBYTES + (b + 1) * HALF,
                        ],
                        start=(k == 0), stop=(k == NBLK // 2 - 1),
                        tile_position=(0, half * DIM),
                    )

            def emit_d(h, piece):
                # D piece = (R*x^T) (.) Y  on the DVE ([128, 256])
                o = piece * PIECE
                nc.vector.scalar_tensor_tensor(
                    d_sbs[h][:, o : o + PIECE],
                    xtr2_sb[:, h * HALF + o : h * HALF + o + PIECE],
                    1.0,
                    ys[h][:, o : o + PIECE],
                    op0=mult, op1=mult,
                )

            def emit_reduce(h, piece):
                # OutP piece -= sum_p D[p, :]  (-1s stationary matmul)
                o = piece * PIECE
                nc.tensor.matmul(
                    outp[:, h * HALF + o : h * HALF + o + PIECE],
                    wneg_sb[:],
                    d_sbs[h][:, o : o + PIECE],
                    start=False, stop=True,
                )

            def emit_out(h, piece):
                # out^T slice = OutP slice + F  (ScalarE), then DMA from
                # the Sync ring (idle at the end; keeps ScalarE free for
                # the next ACT piece).
                o = h * HALF + piece * PIECE
                nc.scalar.activation(
                    o_sb[:, o : o + PIECE],
                    outp[:, o : o + PIECE],
                    mybir.ActivationFunctionType.Copy,
                    bias=F_CONST, scale=1.0,
                )
                nc.sync.dma_start(
                    out=out[:, o : o + PIECE], in_=o_sb[:, o : o + PIECE]
                )

            # Half 0 matmul stream
            for k in range(NBLK // 2):
                emit_y_mm(0, k)
            emit_d(0, 0)
            emit_d(0, 1)
            # Half 1 stream; half 0's reduce+epilogue emitted a few pairs
            # in so the PE queue never drains while the DVE computes D0.
            for k in range(NBLK // 2):
                emit_y_mm(1, k)
                if k == 3:
                    emit_reduce(0, 0)
                if k == 4:
                    emit_reduce(0, 1)
                if k == 5:
                    emit_out(0, 0)
                if k == 7:
                    emit_out(0, 1)
            # Tail: piece-wise so DVE / PE / ScalarE / DMA pipeline.
            emit_d(1, 0)
            emit_reduce(1, 0)
            emit_out(1, 0)
            emit_d(1, 1)
            emit_reduce(1, 1)
            emit_out(1, 1)

    nc.finalize()
    return nc


def _get_nc():
    if "nc" not in _CACHE:
        _CACHE["nc"] = _build_nc()
    return _CACHE["nc"]


def _make_in_maps(x, A):
    import ml_dtypes

    bf16 = ml_dtypes.bfloat16
    f8 = ml_dtypes.float8_e4m3
    x = np.ascontiguousarray(np.asarray(x, dtype=np.float32))
    A = np.ascontiguousarray(np.asarray(A, dtype=np.float32))

    x8 = x.astype(f8)
    # xs[p, 64*b + d] = x8[128*b + p, d]
    xs = np.ascontiguousarray(
        x8.reshape(NBLK, P, DIM).transpose(1, 0, 2)
    ).reshape(P, NBLK * DIM)
    wneg = np.full((P, DIM), -1.0, dtype=bf16)
    wbi = (-B_CONST * np.eye(DIM, dtype=np.float32)).astype(bf16)

    in_maps = []
    for c in range(NCORES):
        rows = slice(c * ROWS, (c + 1) * ROWS)
        a8 = A[rows].astype(f8)  # [1024, 8192]
        # bt[p, h*HBYTES + b*HALF + i'] = a8[HALF*h + i', 128b + p]
        bt = np.ascontiguousarray(
            a8.reshape(NH, HALF, NBLK, P).transpose(3, 0, 2, 1)
        ).reshape(P, NH * HBYTES)
        xloc = x[rows]                      # [1024, 64] f32
        xt = np.ascontiguousarray(xloc.T)   # [64, 1024]
        in_maps.append(
            {
                "bt": bt,
                "xs": xs,
                "xtr": (R_CONST * xt).astype(bf16),
                "xtb": xt.astype(bf16),
                "wneg": wneg,
                "wbi": wbi,
            }
        )
    return in_maps


def run_sharded(x, A, trace=False, **kwargs):
    """Run the SPMD bass kernel; returns (full_output, BassKernelResults)."""
    from concourse.bass_utils import run_bass_kernel_spmd

    nc = _get_nc()
    res = run_bass_kernel_spmd(
        nc, _make_in_maps(x, A), core_ids=list(range(NCORES)), trace=trace, **kwargs
    )
    # out is [64, 1024] bf16 per core -> [1024, 64] f32, concatenated
    full = np.concatenate(
        [res.results[c]["out"].astype(np.float32).T for c in range(NCORES)], axis=0
    )
    return np.ascontiguousarray(full), res


def kernel(t, x, A):
    out, _ = run_sharded(x, A)
    return out


# revision 23
# speedup vs baseline: 1.0838x; 1.0838x over previous
"""Trainium2 Bass kernel for BiochemicalDynamics — DoubleRow variant (v9 tail).

Same algorithm as kernel.py (Y = (A_c @ x)^T on the TensorEngine from
fp8 A^T tiles), but the j-contraction uses fp8 DoubleRow perf mode:
each matmul contracts a 256-row j-superblock ([K=128, Ko=2] 3D APs for
both operands), halving both the matmul count and the LDWEIGHTS count.
No column-tiling (mutually exclusive with DoubleRow), so Y lives in
PSUM partitions 0..63 only and the x^T multiplier needs no duplication.
"""

import sys

import numpy as np

for _p in ("/opt/trn_rl_repo", "/root/.axon_site/_ro/trn_rl_repo"):
    if _p not in sys.path:
        sys.path.append(_p)

N = 8192
DIM = 64
NCORES = 8
ROWS = N // NCORES       # 1024 rows of A (and output) per core

F_CONST = 1.0
B_CONST = 0.1
R_CONST = 0.01

P = 128                  # SBUF partitions
NSB = N // 256           # 32 j-superblocks (256 rows each)
HALF = 512               # output-column half width
NH = ROWS // HALF        # 2 halves
HBYTES = NSB * 2 * HALF  # fp8 bytes per half per partition (32KB)

# bt DMA chunks per half, in j-superblocks (sb-tile = 128KB fp8).
BT_CHUNKS = [
    [2, 2, 4, 4, 4, 4, 4, 4, 4],        # half 0
    [4, 4, 4, 4, 4, 4, 4, 2, 1, 1],     # half 1: ramp down at the end
]
assert all(sum(c) == NSB for c in BT_CHUNKS)

N_WARM = 16              # warm-up matmuls (~3.4us cold) to trip HAM

_CACHE = {}


def _build_nc():
    import concourse.mybir as mybir
    import concourse.tile as tile
    from concourse import bacc

    f32 = mybir.dt.float32
    bf16 = mybir.dt.bfloat16
    f8 = mybir.dt.float8e4
    DR = mybir.MatmulPerfMode.DoubleRow

    nc = bacc.Bacc(
        trn_type="TRN2", target_bir_lowering=False, debug=False, num_devices=NCORES
    )

    # A^T superblocks, fp8:
    # bt[p, h*HBYTES + sb*1024 + i*512 + n] = A[rows_c[512h+n], 256sb+128i+p]
    bt = nc.dram_tensor("bt", [P, NH * HBYTES], f8, kind="ExternalInput")
    # x stationaries, fp8: xs[p, sb*128 + i*64 + d] = x[256sb + 128i + p, d]
    xs = nc.dram_tensor("xs", [P, NSB * 2 * DIM], f8, kind="ExternalInput")
    # R*xloc^T bf16 [64, 1024]
    xtr = nc.dram_tensor("xtr", [DIM, ROWS], bf16, kind="ExternalInput")
    # xloc^T bf16 (moving operand of the -B*x seed matmul)
    xtb = nc.dram_tensor("xtb", [DIM, ROWS], bf16, kind="ExternalInput")
    # -1s [64, 64] and -B*I64, bf16 stationaries
    wneg = nc.dram_tensor("wneg", [DIM, DIM], bf16, kind="ExternalInput")
    wbi = nc.dram_tensor("wbi", [DIM, DIM], bf16, kind="ExternalInput")
    # out^T bf16: out[d, i] = F - B*xloc[i, d] - s_i
    out = nc.dram_tensor("out", [DIM, ROWS], bf16, kind="ExternalOutput")

    mult = mybir.AluOpType.mult

    with tile.TileContext(nc) as tc:
        with (
            tc.tile_pool(name="big", bufs=1) as big,
            tc.tile_pool(name="small", bufs=1) as small,
            tc.tile_pool(name="psum", bufs=1, space="PSUM") as psum_pool,
        ):
            # --- PE warm-up on a memset scratch tile (no input deps) ---
            scr = small.tile([P, 256], f32)
            nc.vector.memset(scr[:], 1.0)
            warm_ps = psum_pool.tile([DIM, 256], f32, tag="warm")
            for _ in range(N_WARM):
                nc.tensor.matmul(
                    warm_ps[:], scr[:, :DIM], scr[:], start=True, stop=True
                )

            # --- x-side loads on the Scalar (ACT) HWDGE ring ---
            wbi_sb = small.tile([DIM, DIM], bf16)
            nc.scalar.dma_start(out=wbi_sb[:], in_=wbi[:])
            xtb_sb = small.tile([DIM, ROWS], bf16)
            nc.scalar.dma_start(out=xtb_sb[:], in_=xtb[:])
            xs_sb = small.tile([P, NSB * 2 * DIM], f8)
            for o, w in ((0, 16 * 128), (16 * 128, 16 * 128)):
                nc.scalar.dma_start(out=xs_sb[:, o : o + w], in_=xs[:, o : o + w])
            xtr_sb = small.tile([DIM, ROWS], bf16)
            nc.scalar.dma_start(out=xtr_sb[:], in_=xtr[:])
            wneg_sb = small.tile([DIM, DIM], bf16)
            nc.scalar.dma_start(out=wneg_sb[:], in_=wneg[:])

            # --- A^T stream, all on the Sync HWDGE ring ---
            bt_sb = big.tile([P, NH * HBYTES], f8)
            for h in range(NH):
                boff = 0
                for nb in BT_CHUNKS[h]:
                    o = h * HBYTES + boff * 1024
                    w = nb * 1024
                    nc.sync.dma_start(out=bt_sb[:, o : o + w], in_=bt[:, o : o + w])
                    boff += nb

            # Output accumulator [64, 1024] f32 (2 PSUM banks).
            outp = psum_pool.tile([DIM, ROWS], f32, tag="outp")
            PIECE = HALF // 2
            for q in range(ROWS // PIECE):
                nc.tensor.matmul(
                    outp[:, q * PIECE : (q + 1) * PIECE],
                    wbi_sb[:],
                    xtb_sb[:, q * PIECE : (q + 1) * PIECE],
                    start=True, stop=False,
                )

            o_sb = small.tile([DIM, ROWS], bf16)
            ys = [
                psum_pool.tile([DIM, HALF], f32, tag=f"y{h}", name=f"y{h}")
                for h in range(NH)
            ]
            d_sbs = [
                small.tile([DIM, HALF], bf16, tag=f"d{h}", name=f"d{h}")
                for h in range(NH)
            ]

            def emit_y_mm(h, sb):
                lhsT = xs_sb[:, sb * 128 : (sb + 1) * 128].rearrange(
                    "p (two d) -> p two d", two=2
                )
                rhs = bt_sb[
                    :, h * HBYTES + sb * 1024 : h * HBYTES + (sb + 1) * 1024
                ].rearrange("p (two n) -> p two n", two=2)
                nc.tensor.matmul(
                    ys[h][:],
                    lhsT,
                    rhs,
                    start=(sb == 0), stop=(sb == NSB - 1),
                    perf_mode=DR,
                )

            def emit_d(h, piece):
                o = piece * PIECE
                nc.vector.scalar_tensor_tensor(
                    d_sbs[h][:, o : o + PIECE],
                    xtr_sb[:, h * HALF + o : h * HALF + o + PIECE],
                    1.0,
                    ys[h][:, o : o + PIECE],
                    op0=mult, op1=mult,
                )

            def emit_reduce(h, piece):
                o = piece * PIECE
                nc.tensor.matmul(
                    outp[:, h * HALF + o : h * HALF + o + PIECE],
                    wneg_sb[:],
                    d_sbs[h][:, o : o + PIECE],
                    start=False, stop=True,
                )

            def emit_out(h, piece):
                o = h * HALF + piece * PIECE
                nc.scalar.activation(
                    o_sb[:, o : o + PIECE],
                    outp[:, o : o + PIECE],
                    mybir.ActivationFunctionType.Copy,
                    bias=F_CONST, scale=1.0,
                )
                nc.sync.dma_start(
                    out=out[:, o : o + PIECE], in_=o_sb[:, o : o + PIECE]
                )

            for sb in range(NSB):
                emit_y_mm(0, sb)
            emit_d(0, 0)
            emit_d(0, 1)
            for sb in range(NSB):
                emit_y_mm(1, sb)
                if sb == 3:
                    emit_reduce(0, 0)
                if sb == 4:
                    emit_reduce(0, 1)
                if sb == 5:
                    emit_out(0, 0)
                if sb == 7:
                    emit_out(0, 1)
            emit_d(1, 0)
            emit_reduce(1, 0)
            emit_out(1, 0)
            emit_d(1, 1)
            emit_reduce(1, 1)
            emit_out(1, 1)

    nc.finalize()
    return nc


def _get_nc():
    if "nc" not in _CACHE:
        _CACHE["nc"] = _build_nc()
    return _CACHE["nc"]


def _make_in_maps(x, A):
    import ml_dtypes

    bf16 = ml_dtypes.bfloat16
    f8 = ml_dtypes.float8_e4m3
    x = np.ascontiguousarray(np.asarray(x, dtype=np.float32))
    A = np.ascontiguousarray(np.asarray(A, dtype=np.float32))

    x8 = x.astype(f8)
    # xs[p, sb*128 + i*64 + d] = x8[256sb + 128i + p, d]
    xs = np.ascontiguousarray(
        x8.reshape(NSB, 2, P, DIM).transpose(2, 0, 1, 3)
    ).reshape(P, NSB * 2 * DIM)
    wneg = np.full((DIM, DIM), -1.0, dtype=bf16)
    wbi = (-B_CONST * np.eye(DIM, dtype=np.float32)).astype(bf16)

    in_maps = []
    for c in range(NCORES):
        rows = slice(c * ROWS, (c + 1) * ROWS)
        a8 = A[rows].astype(f8)  # [1024, 8192]
        # bt[p, h*HBYTES + sb*1024 + i*512 + n] = a8[512h + n, 256sb + 128i + p]
        bt = np.ascontiguousarray(
            a8.reshape(NH, HALF, NSB, 2, P).transpose(4, 0, 2, 3, 1)
        ).reshape(P, NH * HBYTES)
        xloc = x[rows]                      # [1024, 64] f32
        xt = np.ascontiguousarray(xloc.T)   # [64, 1024]
        in_maps.append(
            {
                "bt": bt,
                "xs": xs,
                "xtr": (R_CONST * xt).astype(bf16),
                "xtb": xt.astype(bf16),
                "wneg": wneg,
                "wbi": wbi,
            }
        )
    return in_maps


def run_sharded(x, A, trace=False, **kwargs):
    """Run the SPMD bass kernel; returns (full_output, BassKernelResults)."""
    from concourse.bass_utils import run_bass_kernel_spmd

    nc = _get_nc()
    res = run_bass_kernel_spmd(
        nc, _make_in_maps(x, A), core_ids=list(range(NCORES)), trace=trace, **kwargs
    )
    full = np.concatenate(
        [res.results[c]["out"].astype(np.float32).T for c in range(NCORES)], axis=0
    )
    return np.ascontiguousarray(full), res


def kernel(t, x, A):
    out, _ = run_sharded(x, A)
    return out
